# revision 12
# baseline (speedup 1.0000x reference)
"""DefectAttractor (retrieval KNN) Trainium2 Bass kernel.

Math (per row x of defect_location [N, D], sites s [M, D]):
    nearest = argmin_m ||x - s_m||^2  = argmax_m (x.s_m - 0.5||s_m||^2)
    ricci   = rate * (s[nearest] - x)
    exceeds = |ricci| > cohesion + |x.ricci|/(|x|+eps) * tan(friction)
    out     = ricci * (exceeds ? 2.0 : 0.5)

Device pipeline per 128-row tile (natural layout [n, m], data parallel over
8 cores on the N axis):
  PE:  z = xh.sh + xh.sl + xl.sh + ones3x(bias hi/mid/lo)   (fp16 3-pass
       split matmul, 22-bit-exact products, fp32 PSUM accumulate; abs err
       ~3e-6 which is below the smallest observed argmin gap ~1.1e-5)
  DVE: r = running-max scan over z (PSUM -> SBUF, exact comparisons)
  ACT: cnt = sum_m sigmoid(BETA*(r - maxv) + 18)  == M - argmax index
       (BETA=2^23; saturates exactly to 0/1 away from a ~6e-6 window)
  DMA: indirect gather of s_aug[idx] = [s_m | s2_m | pad] rows
  epilogue: dir = s_near - x (GPSIMD); Mohr-Coulomb scalars batched per
       16-tile chunk on DVE (heron sqrt, no ACT table switch);
       out = dir * (rate*scale)  -- bitwise equal to reference's
       (rate*dir)*scale since scale is a power of two.
"""
import numpy as np
from contextlib import ExitStack

import concourse.bass as bass
import concourse.bacc as bacc
import concourse.tile as tile
import concourse.mybir as mybir
import concourse.bass_utils as bass_utils

N, M, D = 131072, 1024, 128
NCORES = 8
R = N // NCORES            # rows per core
P = 128                    # partitions / tile rows
T = R // P                 # tiles per core (128)
TCH = 32                   # tiles per scalar-math chunk
NCHUNK = T // TCH
BETA = float(2 ** 23)
EPS = 1e-8
E = 192                    # gather row elements (s | s2 | pad), 768B
NEG_BIG = -1e30

f16 = mybir.dt.float16
f32 = mybir.dt.float32
i32 = mybir.dt.int32
u32 = mybir.dt.uint32
Alu = mybir.AluOpType
Act = mybir.ActivationFunctionType

_cache = {}


def _build(rate, coh, tanf, repeat=1):
    nc = bacc.Bacc("TRN2", target_bir_lowering=False, debug=False,
                   num_devices=NCORES)

    xh_d = nc.dram_tensor("xh_t", [P, R], f16, kind="ExternalInput")
    xl_d = nc.dram_tensor("xl_t", [P, R], f16, kind="ExternalInput")
    x_d = nc.dram_tensor("x_nat", [R, P], f32, kind="ExternalInput")
    sh_d = nc.dram_tensor("sh_t", [P, M], f16, kind="ExternalInput")
    sl_d = nc.dram_tensor("sl_t", [P, M], f16, kind="ExternalInput")
    b3_d = nc.dram_tensor("bias3", [4, M], f16, kind="ExternalInput")
    sa_d = nc.dram_tensor("s_aug", [M, E], f32, kind="ExternalInput")
    x2_d = nc.dram_tensor("x2in", [R, 1], f32, kind="ExternalInput")
    out_d = nc.dram_tensor("out", [R, P], f32, kind="ExternalOutput")

    c2v = np.float32(rate) * np.float32(2.0)
    c05v = np.float32(rate) * np.float32(0.5)
    rate2 = np.float32(rate) * np.float32(rate)

    with tile.TileContext(nc) as tc, ExitStack() as ctx:
        const = ctx.enter_context(tc.tile_pool(name="const", bufs=1))
        xw = ctx.enter_context(tc.tile_pool(name="xw", bufs=2))
        xnat = ctx.enter_context(tc.tile_pool(name="xnat", bufs=2))
        zpool = ctx.enter_context(tc.tile_pool(name="zp", bufs=3, space="PSUM"))
        rpool = ctx.enter_context(tc.tile_pool(name="rp", bufs=4))
        junk = ctx.enter_context(tc.tile_pool(name="junk", bufs=3))
        stats = ctx.enter_context(tc.tile_pool(name="stats", bufs=3))
        gpool = ctx.enter_context(tc.tile_pool(name="gp", bufs=TCH + 2))
        dpool = ctx.enter_context(tc.tile_pool(name="dp", bufs=3))
        opool = ctx.enter_context(tc.tile_pool(name="op", bufs=2))
        small = ctx.enter_context(tc.tile_pool(name="small", bufs=4))

        shT = const.tile([P, M], f16)
        slT = const.tile([P, M], f16)
        bias3 = const.tile([4, M], f16)
        ones3 = const.tile([4, 1], f16)
        negb = const.tile([P, 1], f32)
        c2t = const.tile([P, 1], f32)
        c05t = const.tile([P, 1], f32)
        nc.sync.dma_start(shT[:], sh_d.ap())
        nc.sync.dma_start(slT[:], sl_d.ap())
        nc.sync.dma_start(bias3[:], b3_d.ap())
        nc.vector.memset(ones3[:], 1.0)
        nc.vector.memset(negb[:], NEG_BIG)
        nc.vector.memset(c2t[:], float(c2v))
        nc.vector.memset(c05t[:], float(c05v))

        import contextlib
        loop_cm = tc.For_i(0, repeat, 1) if repeat > 1 else contextlib.nullcontext()
        with loop_cm:
            for ch in range(NCHUNK):
                ab_c = stats.tile([P, TCH], f32, tag="abc")
                cnt_c = stats.tile([P, TCH], f32, tag="cnt")
                s2g_c = stats.tile([P, TCH], f32, tag="s2g")
                g_list = []
                ccols = slice(ch * TCH * P, (ch + 1) * TCH * P)
                # one batched DMA per chunk for each input stream
                xh_c = xw.tile([P, TCH * P], f16, tag="xh")
                nc.sync.dma_start(xh_c[:], xh_d.ap()[:, ccols])
                xl_c = xw.tile([P, TCH * P], f16, tag="xl")
                nc.sync.dma_start(xl_c[:], xl_d.ap()[:, ccols])
                x_c = xnat.tile([P, TCH, P], f32, tag="xn")
                nc.sync.dma_start(
                    x_c[:], x_d.ap()[ccols, :].rearrange("(t p) d -> p t d", p=P))
                x2_c = stats.tile([P, TCH], f32, tag="x2")
                nc.sync.dma_start(
                    x2_c[:], x2_d.ap()[ccols, :].rearrange("(t p) o -> p (t o)", p=P))
                for tl in range(TCH):
                    tcols = slice(tl * P, (tl + 1) * P)
                    xh_t = xh_c[:, tcols]
                    xl_t = xl_c[:, tcols]
                    x_t = x_c[:, tl, :]

                    z = zpool.tile([P, M], f32, tag="z")
                    for b in (0, 1):
                        cs = slice(b * 512, (b + 1) * 512)
                        nc.tensor.matmul(z[:, cs], xh_t, shT[:, cs],
                                         start=True, stop=False)
                        nc.tensor.matmul(z[:, cs], xh_t, slT[:, cs],
                                         start=False, stop=False)
                        nc.tensor.matmul(z[:, cs], xl_t, shT[:, cs],
                                         start=False, stop=False)
                        nc.tensor.matmul(z[:, cs],
                                         ones3[:].to_broadcast([4, P]),
                                         bias3[:, cs], start=False, stop=True)

                    r = rpool.tile([P, M], f32, tag="r")
                    nc.vector.tensor_tensor_scan(
                        r[:], z[:], negb[:].to_broadcast([P, M]), NEG_BIG,
                        op0=Alu.max, op1=Alu.max)
                    nc.vector.tensor_scalar(ab_c[:, tl:tl + 1], r[:, M - 1:M],
                                            -BETA, 18.0, op0=Alu.mult,
                                            op1=Alu.add)
                    jk = junk.tile([P, M], f32, tag="jk")
                    nc.scalar.activation(jk[:], r[:], Act.Sigmoid,
                                         bias=ab_c[:, tl:tl + 1], scale=BETA,
                                         accum_out=cnt_c[:, tl:tl + 1])

                maxv_c = stats.tile([P, TCH], f32, tag="maxv")
                nc.vector.tensor_scalar(maxv_c[:], ab_c[:], float(-1.0 / BETA),
                                        float(18.0 / BETA), op0=Alu.mult,
                                        op1=Alu.add)
                # indices for the whole chunk: idx = M - round(cnt)
                idxf = stats.tile([P, TCH], f32, tag="idxf")
                nc.vector.tensor_scalar(idxf[:], cnt_c[:], -1.0, float(M),
                                        op0=Alu.mult, op1=Alu.add)
                idxi = stats.tile([P, TCH], i32, tag="idxi")
                nc.vector.tensor_copy(idxi[:], idxf[:])

                for tl in range(TCH):
                    g = gpool.tile([P, E], f32, tag="g")
                    nc.gpsimd.indirect_dma_start(
                        out=g[:], out_offset=None, in_=sa_d.ap(),
                        in_offset=bass.IndirectOffsetOnAxis(
                            ap=idxi[:, tl:tl + 1], axis=0))
                    g_list.append(g)
                    nc.vector.tensor_copy(s2g_c[:, tl:tl + 1], g[:, 128:129])

                # batched Mohr-Coulomb scalar math for the chunk
                d2m = stats.tile([P, TCH], f32, tag="d2m")
                nc.vector.scalar_tensor_tensor(
                    d2m[:], maxv_c[:], -2.0, x2_c[:], op0=Alu.mult, op1=Alu.add)
                sqin = stats.tile([P, 2 * TCH], f32, tag="sqin")
                nc.vector.tensor_scalar(sqin[:, :TCH], d2m[:], 0.0, None,
                                        op0=Alu.max)
                nc.vector.tensor_copy(sqin[:, TCH:], x2_c[:])
                # heron sqrt, 4 iterations, seed 12
                sq = stats.tile([P, 2 * TCH], f32, tag="sq")
                nc.vector.tensor_scalar(sq[:], sqin[:], 0.09, 4.0,
                                        op0=Alu.mult, op1=Alu.add)
                for _ in range(3):
                    recs = stats.tile([P, 2 * TCH], f32, tag="recs")
                    nc.vector.reciprocal(recs[:], sq[:])
                    quot = stats.tile([P, 2 * TCH], f32, tag="quot")
                    nc.vector.tensor_tensor(quot[:], sqin[:], recs[:],
                                            op=Alu.mult)
                    nc.vector.tensor_tensor(sq[:], sq[:], quot[:], op=Alu.add)
                    nc.vector.tensor_scalar(sq[:], sq[:], 0.5, None,
                                            op0=Alu.mult)
                stress = stats.tile([P, TCH], f32, tag="stress")
                nc.vector.tensor_scalar(stress[:], sq[:, :TCH], float(rate),
                                        None, op0=Alu.mult)
                # x.ricci = rate*(maxv + 0.5*s2g - x2)
                t1 = stats.tile([P, TCH], f32, tag="t1")
                nc.vector.scalar_tensor_tensor(
                    t1[:], s2g_c[:], 0.5, maxv_c[:], op0=Alu.mult, op1=Alu.add)
                t2 = stats.tile([P, TCH], f32, tag="t2")
                nc.vector.tensor_tensor(t2[:], t1[:], x2_c[:], op=Alu.subtract)
                xr = stats.tile([P, TCH], f32, tag="xr")
                nc.vector.tensor_scalar(xr[:], t2[:], float(rate), None,
                                        op0=Alu.mult)
                axr = stats.tile([P, TCH], f32, tag="axr")
                nc.scalar.activation(axr[:], xr[:], Act.Abs)
                den = stats.tile([P, TCH], f32, tag="den")
                nc.vector.tensor_scalar(den[:], sq[:, TCH:], float(EPS), None,
                                        op0=Alu.add)
                rden = stats.tile([P, TCH], f32, tag="rden")
                nc.vector.reciprocal(rden[:], den[:])
                normal = stats.tile([P, TCH], f32, tag="normal")
                nc.vector.tensor_tensor(normal[:], axr[:], rden[:], op=Alu.mult)
                yld = stats.tile([P, TCH], f32, tag="yld")
                nc.vector.tensor_scalar(yld[:], normal[:], float(tanf),
                                        float(coh), op0=Alu.mult, op1=Alu.add)
                exc = stats.tile([P, TCH], u32, tag="exc")
                nc.vector.tensor_tensor(exc[:], stress[:], yld[:], op=Alu.is_gt)
                coef = stats.tile([P, TCH], f32, tag="coef")
                nc.vector.select(coef[:], exc[:],
                                 c2t[:].to_broadcast([P, TCH]),
                                 c05t[:].to_broadcast([P, TCH]))

                out_ch = opool.tile([P, TCH, P], f32, tag="out")
                for tl in range(TCH):
                    dir_t = dpool.tile([P, P], f32, tag="dir")
                    nc.gpsimd.tensor_tensor(dir_t[:], g_list[tl][:, :128],
                                            x_c[:, tl, :], op=Alu.subtract)
                    nc.vector.tensor_scalar(out_ch[:, tl, :], dir_t[:],
                                            coef[:, tl:tl + 1], None,
                                            op0=Alu.mult)
                nc.sync.dma_start(
                    out_d.ap()[ccols, :].rearrange("(t p) d -> p t d", p=P),
                    out_ch[:])

    nc.compile()
    return nc


def _prep(x, s):
    """Host-side input prep shared across cores."""
    xT = np.ascontiguousarray(x.T)                       # [D, N] fp32
    xh = xT.astype(np.float16)
    xl = (xT - xh.astype(np.float32)).astype(np.float16)

    sT = np.ascontiguousarray(s.T)                       # [D, M]
    sh = sT.astype(np.float16)
    sl = (sT - sh.astype(np.float32)).astype(np.float16)

    s2_64 = (s.astype(np.float64) ** 2).sum(1)
    bias = -0.5 * s2_64
    b1 = bias.astype(np.float16)
    b2 = (bias - b1.astype(np.float64)).astype(np.float16)
    b3 = (bias - b1.astype(np.float64) - b2.astype(np.float64)).astype(np.float16)
    bias3 = np.zeros((4, M), np.float16)
    bias3[0], bias3[1], bias3[2] = b1, b2, b3

    s_aug = np.zeros((M, E), np.float32)
    s_aug[:, :D] = s
    s_aug[:, D] = s2_64.astype(np.float32)
    x2 = (x.astype(np.float64) ** 2).sum(1).astype(np.float32)[:, None]
    return xh, xl, sh, sl, bias3, s_aug, x2


def kernel(**inputs):
    x = np.ascontiguousarray(np.asarray(inputs["defect_location"], dtype=np.float32))
    s = np.ascontiguousarray(np.asarray(inputs["defect_sites"], dtype=np.float32))
    rate = float(np.asarray(inputs["ricci_flow_rate"]).reshape(-1)[0])
    coh = float(np.asarray(inputs["cohesion"]).reshape(-1)[0])
    fric = float(np.asarray(inputs["friction_angle"]).reshape(-1)[0])
    tanf = float(np.float32(np.tan(np.float64(np.float32(fric)))))

    xh, xl, sh, sl, bias3, s_aug, x2 = _prep(x, s)

    key = (rate, coh, fric)
    if key not in _cache:
        _cache[key] = _build(rate, coh, tanf)
    nc = _cache[key]

    in_maps = []
    for c in range(NCORES):
        cols = slice(c * R, (c + 1) * R)
        in_maps.append({
            "xh_t": np.ascontiguousarray(xh[:, cols]),
            "xl_t": np.ascontiguousarray(xl[:, cols]),
            "x_nat": np.ascontiguousarray(x[cols, :]),
            "x2in": np.ascontiguousarray(x2[cols, :]),
            "sh_t": sh,
            "sl_t": sl,
            "bias3": bias3,
            "s_aug": s_aug,
        })

    res = bass_utils.run_bass_kernel_spmd(nc, in_maps,
                                          core_ids=list(range(NCORES)))
    out = np.concatenate([res.results[c]["out"] for c in range(NCORES)], axis=0)
    return out


if __name__ == "__main__":
    import time
    x = np.load("/tmp/x.npy")
    s = np.load("/tmp/s.npy")
    rate, coh, fric = np.load("/tmp/scalars.npy")
    t0 = time.time()
    out = kernel(defect_location=x, defect_sites=s,
                 ricci_flow_rate=np.float32(rate), cohesion=np.float32(coh),
                 friction_angle=np.float32(fric))
    print("kernel wall:", time.time() - t0)
    exp = np.load("/tmp/expected.npy")
    err = np.abs(out - exp)
    rel = np.linalg.norm((out - exp).astype(np.float64)) / np.linalg.norm(exp.astype(np.float64))
    print("absmax err:", err.max(), "rel l2:", rel)
    bad_rows = (err.max(1) > 1e-4).sum()
    print("rows with absmax>1e-4:", bad_rows)


# revision 13
# speedup vs baseline: 1.0125x; 1.0125x over previous
"""DefectAttractor (retrieval KNN) Trainium2 Bass kernel.

Math (per row x of defect_location [N, D], sites s [M, D]):
    nearest = argmin_m ||x - s_m||^2  = argmax_m (x.s_m - 0.5||s_m||^2)
    ricci   = rate * (s[nearest] - x)
    exceeds = |ricci| > cohesion + |x.ricci|/(|x|+eps) * tan(friction)
    out     = ricci * (exceeds ? 2.0 : 0.5)

Device pipeline per 128-row tile (natural layout [n, m], data parallel over
8 cores on the N axis):
  PE:  z = xh.sh + xh.sl + xl.sh + ones3x(bias hi/mid/lo)   (fp16 3-pass
       split matmul, 22-bit-exact products, fp32 PSUM accumulate; abs err
       ~3e-6 which is below the smallest observed argmin gap ~1.1e-5)
  DVE: r = running-max scan over z (PSUM -> SBUF, exact comparisons)
  ACT: cnt = sum_m sigmoid(BETA*(r - maxv) + 18)  == M - argmax index
       (BETA=2^23; saturates exactly to 0/1 away from a ~6e-6 window)
  DMA: indirect gather of s_aug[idx] = [s_m | s2_m | pad] rows
  epilogue: dir = s_near - x (GPSIMD); Mohr-Coulomb scalars batched per
       16-tile chunk on DVE (heron sqrt, no ACT table switch);
       out = dir * (rate*scale)  -- bitwise equal to reference's
       (rate*dir)*scale since scale is a power of two.
"""
import numpy as np
from contextlib import ExitStack

import concourse.bass as bass
import concourse.bacc as bacc
import concourse.tile as tile
import concourse.mybir as mybir
import concourse.bass_utils as bass_utils

N, M, D = 131072, 1024, 128
NCORES = 8
R = N // NCORES            # rows per core
P = 128                    # partitions / tile rows
T = R // P                 # tiles per core (128)
TCH = 32                   # tiles per scalar-math chunk
NCHUNK = T // TCH
BETA = float(2 ** 23)
EPS = 1e-8
E = 192                    # gather row elements (s | s2 | pad), 768B
NEG_BIG = -1e30

f16 = mybir.dt.float16
f32 = mybir.dt.float32
i32 = mybir.dt.int32
u32 = mybir.dt.uint32
Alu = mybir.AluOpType
Act = mybir.ActivationFunctionType

_cache = {}


def _build(rate, coh, tanf, repeat=1):
    nc = bacc.Bacc("TRN2", target_bir_lowering=False, debug=False,
                   num_devices=NCORES)

    xh_d = nc.dram_tensor("xh_t", [P, R], f16, kind="ExternalInput")
    xl_d = nc.dram_tensor("xl_t", [P, R], f16, kind="ExternalInput")
    x_d = nc.dram_tensor("x_nat", [R, P], f32, kind="ExternalInput")
    sh_d = nc.dram_tensor("sh_t", [P, M], f16, kind="ExternalInput")
    sl_d = nc.dram_tensor("sl_t", [P, M], f16, kind="ExternalInput")
    b3_d = nc.dram_tensor("bias3", [4, M], f16, kind="ExternalInput")
    sa_d = nc.dram_tensor("s_aug", [M, E], f32, kind="ExternalInput")
    x2_d = nc.dram_tensor("x2in", [R, 1], f32, kind="ExternalInput")
    out_d = nc.dram_tensor("out", [R, P], f32, kind="ExternalOutput")

    c2v = np.float32(rate) * np.float32(2.0)
    c05v = np.float32(rate) * np.float32(0.5)
    rate2 = np.float32(rate) * np.float32(rate)

    with tile.TileContext(nc) as tc, ExitStack() as ctx:
        const = ctx.enter_context(tc.tile_pool(name="const", bufs=1))
        xw = ctx.enter_context(tc.tile_pool(name="xw", bufs=2))
        xnat = ctx.enter_context(tc.tile_pool(name="xnat", bufs=2))
        zpool = ctx.enter_context(tc.tile_pool(name="zp", bufs=3, space="PSUM"))
        rpool = ctx.enter_context(tc.tile_pool(name="rp", bufs=4))
        junk = ctx.enter_context(tc.tile_pool(name="junk", bufs=3))
        stats = ctx.enter_context(tc.tile_pool(name="stats", bufs=3))
        gpool = ctx.enter_context(tc.tile_pool(name="gp", bufs=TCH + 2))
        dpool = ctx.enter_context(tc.tile_pool(name="dp", bufs=3))
        opool = ctx.enter_context(tc.tile_pool(name="op", bufs=2))
        small = ctx.enter_context(tc.tile_pool(name="small", bufs=4))

        shT = const.tile([P, M], f16)
        slT = const.tile([P, M], f16)
        bias3 = const.tile([4, M], f16)
        ones3 = const.tile([4, 1], f16)
        negb = const.tile([P, 1], f32)
        c2t = const.tile([P, 1], f32)
        c05t = const.tile([P, 1], f32)
        nc.sync.dma_start(shT[:], sh_d.ap())
        nc.sync.dma_start(slT[:], sl_d.ap())
        nc.sync.dma_start(bias3[:], b3_d.ap())
        nc.vector.memset(ones3[:], 1.0)
        nc.vector.memset(negb[:], NEG_BIG)
        nc.vector.memset(c2t[:], float(c2v))
        nc.vector.memset(c05t[:], float(c05v))

        import contextlib
        loop_cm = tc.For_i(0, repeat, 1) if repeat > 1 else contextlib.nullcontext()
        with loop_cm:
            for ch in range(NCHUNK):
                ab_c = stats.tile([P, TCH], f32, tag="abc")
                cnt_c = stats.tile([P, TCH], f32, tag="cnt")
                s2g_c = stats.tile([P, TCH], f32, tag="s2g")
                g_list = []
                ccols = slice(ch * TCH * P, (ch + 1) * TCH * P)
                # one batched DMA per chunk for each input stream
                xh_c = xw.tile([P, TCH * P], f16, tag="xh")
                nc.sync.dma_start(xh_c[:], xh_d.ap()[:, ccols])
                xl_c = xw.tile([P, TCH * P], f16, tag="xl")
                nc.sync.dma_start(xl_c[:], xl_d.ap()[:, ccols])
                x_c = xnat.tile([P, TCH, P], f32, tag="xn")
                nc.sync.dma_start(
                    x_c[:], x_d.ap()[ccols, :].rearrange("(t p) d -> p t d", p=P))
                x2_c = stats.tile([P, TCH], f32, tag="x2")
                nc.sync.dma_start(
                    x2_c[:], x2_d.ap()[ccols, :].rearrange("(t p) o -> p (t o)", p=P))
                for tl in range(TCH):
                    tcols = slice(tl * P, (tl + 1) * P)
                    xh_t = xh_c[:, tcols]
                    xl_t = xl_c[:, tcols]
                    x_t = x_c[:, tl, :]

                    z = zpool.tile([P, M], f32, tag="z")
                    b0 = slice(0, 512)
                    b1 = slice(512, 1024)
                    nc.tensor.matmul(z[:, b0], xh_t, shT[:, b0], start=True, stop=False)
                    nc.tensor.matmul(z[:, b1], xh_t, shT[:, b1], start=True, stop=False)
                    nc.tensor.matmul(z[:, b0], xh_t, slT[:, b0], start=False, stop=False)
                    nc.tensor.matmul(z[:, b1], xh_t, slT[:, b1], start=False, stop=False)
                    nc.tensor.matmul(z[:, b0], xl_t, shT[:, b0], start=False, stop=False)
                    nc.tensor.matmul(z[:, b1], xl_t, shT[:, b1], start=False, stop=False)
                    nc.tensor.matmul(z[:, b0], ones3[:].to_broadcast([4, P]),
                                     bias3[:, b0], start=False, stop=True)
                    nc.tensor.matmul(z[:, b1], ones3[:].to_broadcast([4, P]),
                                     bias3[:, b1], start=False, stop=True)

                    r = rpool.tile([P, M], f32, tag="r")
                    nc.vector.tensor_tensor_scan(
                        r[:], z[:], negb[:].to_broadcast([P, M]), NEG_BIG,
                        op0=Alu.max, op1=Alu.max)
                    nc.vector.tensor_scalar(ab_c[:, tl:tl + 1], r[:, M - 1:M],
                                            -BETA, 18.0, op0=Alu.mult,
                                            op1=Alu.add)
                    jk = junk.tile([P, M], f32, tag="jk")
                    nc.scalar.activation(jk[:], r[:], Act.Sigmoid,
                                         bias=ab_c[:, tl:tl + 1], scale=BETA,
                                         accum_out=cnt_c[:, tl:tl + 1])

                maxv_c = stats.tile([P, TCH], f32, tag="maxv")
                nc.vector.tensor_scalar(maxv_c[:], ab_c[:], float(-1.0 / BETA),
                                        float(18.0 / BETA), op0=Alu.mult,
                                        op1=Alu.add)
                # indices for the whole chunk: idx = M - round(cnt)
                idxf = stats.tile([P, TCH], f32, tag="idxf")
                nc.vector.tensor_scalar(idxf[:], cnt_c[:], -1.0, float(M),
                                        op0=Alu.mult, op1=Alu.add)
                idxi = stats.tile([P, TCH], i32, tag="idxi")
                nc.vector.tensor_copy(idxi[:], idxf[:])

                for tl in range(TCH):
                    g = gpool.tile([P, E], f32, tag="g")
                    nc.gpsimd.indirect_dma_start(
                        out=g[:], out_offset=None, in_=sa_d.ap(),
                        in_offset=bass.IndirectOffsetOnAxis(
                            ap=idxi[:, tl:tl + 1], axis=0))
                    g_list.append(g)
                    nc.vector.tensor_copy(s2g_c[:, tl:tl + 1], g[:, 128:129])

                # batched Mohr-Coulomb scalar math for the chunk
                d2m = stats.tile([P, TCH], f32, tag="d2m")
                nc.vector.scalar_tensor_tensor(
                    d2m[:], maxv_c[:], -2.0, x2_c[:], op0=Alu.mult, op1=Alu.add)
                sqin = stats.tile([P, 2 * TCH], f32, tag="sqin")
                nc.vector.tensor_scalar(sqin[:, :TCH], d2m[:], 0.0, None,
                                        op0=Alu.max)
                nc.vector.tensor_copy(sqin[:, TCH:], x2_c[:])
                # heron sqrt, 4 iterations, seed 12
                sq = stats.tile([P, 2 * TCH], f32, tag="sq")
                nc.vector.tensor_scalar(sq[:], sqin[:], 0.09, 4.0,
                                        op0=Alu.mult, op1=Alu.add)
                for _ in range(3):
                    recs = stats.tile([P, 2 * TCH], f32, tag="recs")
                    nc.vector.reciprocal(recs[:], sq[:])
                    quot = stats.tile([P, 2 * TCH], f32, tag="quot")
                    nc.vector.tensor_tensor(quot[:], sqin[:], recs[:],
                                            op=Alu.mult)
                    nc.vector.tensor_tensor(sq[:], sq[:], quot[:], op=Alu.add)
                    nc.vector.tensor_scalar(sq[:], sq[:], 0.5, None,
                                            op0=Alu.mult)
                stress = stats.tile([P, TCH], f32, tag="stress")
                nc.vector.tensor_scalar(stress[:], sq[:, :TCH], float(rate),
                                        None, op0=Alu.mult)
                # x.ricci = rate*(maxv + 0.5*s2g - x2)
                t1 = stats.tile([P, TCH], f32, tag="t1")
                nc.vector.scalar_tensor_tensor(
                    t1[:], s2g_c[:], 0.5, maxv_c[:], op0=Alu.mult, op1=Alu.add)
                t2 = stats.tile([P, TCH], f32, tag="t2")
                nc.vector.tensor_tensor(t2[:], t1[:], x2_c[:], op=Alu.subtract)
                xr = stats.tile([P, TCH], f32, tag="xr")
                nc.vector.tensor_scalar(xr[:], t2[:], float(rate), None,
                                        op0=Alu.mult)
                axr = stats.tile([P, TCH], f32, tag="axr")
                nc.scalar.activation(axr[:], xr[:], Act.Abs)
                den = stats.tile([P, TCH], f32, tag="den")
                nc.vector.tensor_scalar(den[:], sq[:, TCH:], float(EPS), None,
                                        op0=Alu.add)
                rden = stats.tile([P, TCH], f32, tag="rden")
                nc.vector.reciprocal(rden[:], den[:])
                normal = stats.tile([P, TCH], f32, tag="normal")
                nc.vector.tensor_tensor(normal[:], axr[:], rden[:], op=Alu.mult)
                yld = stats.tile([P, TCH], f32, tag="yld")
                nc.vector.tensor_scalar(yld[:], normal[:], float(tanf),
                                        float(coh), op0=Alu.mult, op1=Alu.add)
                exc = stats.tile([P, TCH], u32, tag="exc")
                nc.vector.tensor_tensor(exc[:], stress[:], yld[:], op=Alu.is_gt)
                coef = stats.tile([P, TCH], f32, tag="coef")
                nc.vector.select(coef[:], exc[:],
                                 c2t[:].to_broadcast([P, TCH]),
                                 c05t[:].to_broadcast([P, TCH]))

                out_ch = opool.tile([P, TCH, P], f32, tag="out")
                for tl in range(TCH):
                    dir_t = dpool.tile([P, P], f32, tag="dir")
                    nc.gpsimd.tensor_tensor(dir_t[:], g_list[tl][:, :128],
                                            x_c[:, tl, :], op=Alu.subtract)
                    nc.vector.tensor_scalar(out_ch[:, tl, :], dir_t[:],
                                            coef[:, tl:tl + 1], None,
                                            op0=Alu.mult)
                nc.sync.dma_start(
                    out_d.ap()[ccols, :].rearrange("(t p) d -> p t d", p=P),
                    out_ch[:])

    nc.compile()
    return nc


def _prep(x, s):
    """Host-side input prep shared across cores."""
    xT = np.ascontiguousarray(x.T)                       # [D, N] fp32
    xh = xT.astype(np.float16)
    xl = (xT - xh.astype(np.float32)).astype(np.float16)

    sT = np.ascontiguousarray(s.T)                       # [D, M]
    sh = sT.astype(np.float16)
    sl = (sT - sh.astype(np.float32)).astype(np.float16)

    s2_64 = (s.astype(np.float64) ** 2).sum(1)
    bias = -0.5 * s2_64
    b1 = bias.astype(np.float16)
    b2 = (bias - b1.astype(np.float64)).astype(np.float16)
    b3 = (bias - b1.astype(np.float64) - b2.astype(np.float64)).astype(np.float16)
    bias3 = np.zeros((4, M), np.float16)
    bias3[0], bias3[1], bias3[2] = b1, b2, b3

    s_aug = np.zeros((M, E), np.float32)
    s_aug[:, :D] = s
    s_aug[:, D] = s2_64.astype(np.float32)
    x2 = (x.astype(np.float64) ** 2).sum(1).astype(np.float32)[:, None]
    return xh, xl, sh, sl, bias3, s_aug, x2


def kernel(**inputs):
    x = np.ascontiguousarray(np.asarray(inputs["defect_location"], dtype=np.float32))
    s = np.ascontiguousarray(np.asarray(inputs["defect_sites"], dtype=np.float32))
    rate = float(np.asarray(inputs["ricci_flow_rate"]).reshape(-1)[0])
    coh = float(np.asarray(inputs["cohesion"]).reshape(-1)[0])
    fric = float(np.asarray(inputs["friction_angle"]).reshape(-1)[0])
    tanf = float(np.float32(np.tan(np.float64(np.float32(fric)))))

    xh, xl, sh, sl, bias3, s_aug, x2 = _prep(x, s)

    key = (rate, coh, fric)
    if key not in _cache:
        _cache[key] = _build(rate, coh, tanf)
    nc = _cache[key]

    in_maps = []
    for c in range(NCORES):
        cols = slice(c * R, (c + 1) * R)
        in_maps.append({
            "xh_t": np.ascontiguousarray(xh[:, cols]),
            "xl_t": np.ascontiguousarray(xl[:, cols]),
            "x_nat": np.ascontiguousarray(x[cols, :]),
            "x2in": np.ascontiguousarray(x2[cols, :]),
            "sh_t": sh,
            "sl_t": sl,
            "bias3": bias3,
            "s_aug": s_aug,
        })

    res = bass_utils.run_bass_kernel_spmd(nc, in_maps,
                                          core_ids=list(range(NCORES)))
    out = np.concatenate([res.results[c]["out"] for c in range(NCORES)], axis=0)
    return out


if __name__ == "__main__":
    import time
    x = np.load("/tmp/x.npy")
    s = np.load("/tmp/s.npy")
    rate, coh, fric = np.load("/tmp/scalars.npy")
    t0 = time.time()
    out = kernel(defect_location=x, defect_sites=s,
                 ricci_flow_rate=np.float32(rate), cohesion=np.float32(coh),
                 friction_angle=np.float32(fric))
    print("kernel wall:", time.time() - t0)
    exp = np.load("/tmp/expected.npy")
    err = np.abs(out - exp)
    rel = np.linalg.norm((out - exp).astype(np.float64)) / np.linalg.norm(exp.astype(np.float64))
    print("absmax err:", err.max(), "rel l2:", rel)
    bad_rows = (err.max(1) > 1e-4).sum()
    print("rows with absmax>1e-4:", bad_rows)
